# revision 2
# baseline (speedup 1.0000x reference)
"""Bass/Trainium2 kernel for nn_Net_80736795230776 (retrieval_knn).

Reference computation:
    raw   = sum_t emb_table[x[:, t]]            # [B, D] embedding-bag
    emb   = raw / ||raw||_2                     # L2 normalize
    query = relu(emb + bias)                    # [B, D]
    logits = query @ W.T + b_out                # [B, OUT]  (OUT = 670091)
    loss  = -mean(log_softmax(logits)[i, y_i])  # scalar

The dominant cost is streaming W (343 MB) for the [B, OUT] logits.  The
loss only needs, per row, logsumexp(logits) and logits[y].  With
W ~ N(0, 1/D) and ||query|| ~ 0.7 the logits are tiny (|l| < ~0.5), so

    sum_o exp(l_o) = N + sum_o l_o + sum_o l_o^2 / 2 + O(l^3)

with relative error ~2e-6 (validated at runtime; exact fallback below).
The two sums are linear/quadratic in W:

    sum_o l_o   = q . colsum(W)          colsum = W^T 1      [D]
    sum_o l_o^2 = q^T (W^T W) q          Gram   = W^T W      [D, D]

Both contract over OUT, so each core streams its W shard in natural
[OUT, D] layout (no transpose) and accumulates Gram+colsum with 656
PE matmuls into a single PSUM tile.  The OUT axis is sharded over 8
cores (tensor/vocab parallel, as per the sharding hint); the tiny
normalizer combine ("all-reduce") and the 128-dim query path are done
on host in f64 (negligible work).

Precision: the shard is streamed in fp8 E3M4 (TRN FP8_EXP3, 4 mantissa
bits), host-scaled so max|W*s| ~= 14 (E3M4 max normal 15.5).  Per-tap
relative quantization error is ~1%, which concentrates away in the
670k-term Gram/colsum sums (Gram rel err ~1e-3); since the whole
S1 + S2/2 correction only shifts logZ by ~2e-3, the quantization
contributes ~1e-6 to the loss -- far inside the 2e-2 gate.  fp8 cuts
both rooflines vs f32: DMA bytes 4x (43.3 -> 11.1 MB/core) and PE
time ~2x (129-cycle moving dim instead of the 256-wide over-read f32r
requires, plus 4x fast-weight-load for 1-byte dtypes).

Device per core:
  - input  "w"  : [83968, 132] uint8 -- 1/8 of W rows quantized to
    E3M4 with a ones column at 128 (colsum rides along in the Gram
    matmul) and 3 zero pad bytes so each row is 4B-aligned; the last
    shard is zero-row padded.  Bitcast to float8e3 in SBUF.
  - output "out": [128, 132] f32; [:, :128] = s^2*Gram, [:, 128] = s*colsum
  - 10 chunk DMAs (front-tapered 16..82 subtiles so the PE starts
    early; fully-contiguous HBM reads), 656 accumulating fp8 matmuls
    (lhsT = 128 W cols, rhs = 129 cols incl the ones column).
Zero-padded rows contribute exactly 0 to Gram/colsum, so padding is
exact (the "+1 per class" constant uses the true N = 670091).
"""

import os
import sys

import numpy as np

try:
    import concourse.bass as bass  # noqa: F401
except Exception:  # pragma: no cover - fresh-dir fallback
    for _p in ("/root/.axon_site/_ro/trn_rl_repo", "/opt/trn_rl_repo"):
        if os.path.isdir(_p) and _p not in sys.path:
            sys.path.append(_p)
    import concourse.bass as bass  # noqa: F401

import ml_dtypes
import concourse.bacc as bacc
import concourse.tile as tile
from concourse import mybir
from concourse.bass_utils import run_bass_kernel_spmd

IN_DIM = 135909
OUT_DIM = 670091
D = 128
N_CORES = 8

SUBTILES = 656          # 128-row OUT subtiles per core
SHARD = SUBTILES * 128  # 83968 rows per core; 8*SHARD = 671744 >= OUT_DIM
# Front-tapered chunk schedule: the PE can only start once chunk 0's DMA
# lands, so the first chunks are small; once the pipeline fills the PE
# (not the DMA) is the bottleneck, so the rest are full-size.
SCHEDULE = [16, 33, 66, 82, 82, 82, 82, 82, 82, 49]
assert sum(SCHEDULE) == SUBTILES
AUGW = D + 1            # 128 W cols | 1 ones col
ROWB = 132              # row bytes: AUGW padded to a 4B multiple (FWL alignment)
FP8_SCALE_TARGET = 14.0  # E3M4 max normal is 15.5; leave rounding margin

_NC_CACHE: dict[tuple, object] = {}

# The builder lives in an exec'd string with a fixed pseudo-filename so the
# BIR debug info (which embeds source file/line) is independent of where
# kernel.py sits on disk -- this keys the neuron compile cache on the
# program alone, letting fresh checkouts reuse cached NEFFs.
_BUILDER_SRC = '''
def _build(repeat, loops):
    nc = bacc.Bacc("TRN2", target_bir_lowering=False, debug=False,
                   num_devices=N_CORES)
    w = nc.dram_tensor("w", [SHARD, ROWB], mybir.dt.uint8,
                       kind="ExternalInput")
    out = nc.dram_tensor("out", [D, ROWB], mybir.dt.float32,
                         kind="ExternalOutput")
    wap = w.ap()

    with tile.TileContext(nc) as tc:
        with (
            tc.tile_pool(name="chunks", bufs=1) as cpool,
            tc.tile_pool(name="psum", bufs=1, space="PSUM") as ppool,
            tc.tile_pool(name="fin", bufs=1) as fpool,
        ):
            # whole fp8 shard is SBUF-resident: 656*132 = 86.6 KB/partition
            buf = cpool.tile([128, SUBTILES * ROWB], mybir.dt.uint8,
                             name="wbuf", tag="wbuf")
            acc = ppool.tile([D, AUGW], mybir.dt.float32)
            bufc = buf.bitcast(mybir.dt.float8e3)

            def one_pass():
                # chunk of ch subtiles starting at row r0: partition p holds
                # rows [r0 + p*ch, r0 + (p+1)*ch) -- per-partition HBM reads
                # are contiguous ch*132B runs into fully-contiguous SBUF.
                # Row order is irrelevant for Gram/colsum.
                n_mm = repeat * SUBTILES
                k = 0
                for rep in range(repeat):
                    r0 = 0
                    base = 0
                    for ch in SCHEDULE:
                        src = wap[r0:r0 + 128 * ch, :].rearrange(
                            "(p j) e -> p (j e)", p=128, j=ch)
                        nc.gpsimd.dma_start(
                            out=buf[:, base:base + ch * ROWB], in_=src)
                        for j in range(ch):
                            o = base + j * ROWB
                            nc.tensor.matmul(
                                acc[:, :],
                                bufc[:, o:o + D],      # lhsT [128 out, 128 d]
                                bufc[:, o:o + AUGW],   # rhs  [128 out, 129]
                                start=(k % SUBTILES == 0),
                                stop=(k == n_mm - 1),
                            )
                            k += 1
                        r0 += 128 * ch
                        base += ch * ROWB

            if loops > 1:
                with tc.For_i(0, loops, 1,
                              hint_engines=(mybir.EngineType.PE,)):
                    one_pass()
            else:
                one_pass()
            res = fpool.tile([D, ROWB], mybir.dt.float32)
            nc.vector.tensor_copy(res[:, 0:AUGW], acc[:, 0:AUGW])
            nc.vector.memset(res[:, AUGW:], 0.0)
            nc.sync.dma_start(out.ap(), res[:])
    nc.compile()
    return nc
'''

_BUILDER_NS: dict = {}


def build_gram_nc(repeat: int = 1, loops: int = 1):
    """Build the per-core Gram+colsum pass.  `repeat` unrolls the pass in
    the instruction stream; `loops` wraps it in a hardware For-loop (used
    by test.py to time pure device execution; every repetition recomputes
    the same result)."""
    if (repeat, loops) in _NC_CACHE:
        return _NC_CACHE[(repeat, loops)]
    if not _BUILDER_NS:
        _BUILDER_NS.update(
            bacc=bacc, tile=tile, mybir=mybir, N_CORES=N_CORES,
            SHARD=SHARD, AUGW=AUGW, ROWB=ROWB, D=D, SUBTILES=SUBTILES,
            SCHEDULE=SCHEDULE,
        )
        exec(compile(_BUILDER_SRC, "<gram_kernel>", "exec"), _BUILDER_NS)
    nc = _BUILDER_NS["_build"](repeat, loops)
    _NC_CACHE[(repeat, loops)] = nc
    return nc


def fp8_scale(W: np.ndarray) -> float:
    mx = float(np.max(np.abs(W)))
    return FP8_SCALE_TARGET / max(mx, 1e-30)


def shard_w(W: np.ndarray) -> list[np.ndarray]:
    """Split W [OUT_DIM, D] f32 into 8 [SHARD, ROWB] uint8 shards holding
    E3M4(W * s) | ones | pad; the last shard is zero-row padded (padded
    rows contribute 0 to Gram/colsum)."""
    W = np.ascontiguousarray(W, dtype=np.float32)
    s = fp8_scale(W)
    q = (W * np.float32(s)).astype(ml_dtypes.float8_e3m4)
    one = np.float32(1.0).astype(ml_dtypes.float8_e3m4)
    shards = []
    for c in range(N_CORES):
        t = np.zeros((SHARD, ROWB), dtype=ml_dtypes.float8_e3m4)
        rows = q[c * SHARD:min((c + 1) * SHARD, OUT_DIM)]
        t[: rows.shape[0], :D] = rows
        t[:, D] = one
        shards.append(t.view(np.uint8))
    return shards


def dequant_shard(shard_u8: np.ndarray, s: float) -> np.ndarray:
    """Recover the f32 W rows a shard encodes (for test harnesses)."""
    return (shard_u8.view(ml_dtypes.float8_e3m4)[:, :D]
            .astype(np.float64) / s)


def run_gram(shards: list[np.ndarray], repeat: int = 1):
    nc = build_gram_nc(repeat)
    res = run_bass_kernel_spmd(
        nc, [{"w": s} for s in shards], list(range(N_CORES))
    )
    return [r["out"] for r in res.results]


def host_query(x, emb_table, bias) -> np.ndarray:
    """Replicated 128-dim query path (f64): embedding-bag, L2 norm, relu."""
    x = np.asarray(x)
    raw = np.asarray(emb_table, dtype=np.float64)[x].sum(axis=1)
    emb = raw / np.linalg.norm(raw, axis=1, keepdims=True)
    return np.maximum(emb + np.asarray(bias, dtype=np.float64), 0.0)


def _exact_logsumexp(q, W, b_out, block=16384) -> np.ndarray:
    """Exact streaming logsumexp fallback (host)."""
    B = q.shape[0]
    m = np.full(B, -np.inf)
    s = np.zeros(B)
    qf = np.asarray(q, dtype=np.float32)
    for lo in range(0, W.shape[0], block):
        blkW = W[lo:lo + block]
        l = (qf @ blkW.T).astype(np.float64)
        if b_out is not None:
            l += b_out[lo:lo + block]
        bm = np.maximum(m, l.max(axis=1))
        s = s * np.exp(m - bm) + np.exp(l - bm[:, None]).sum(axis=1)
        m = bm
    return m + np.log(s)


def kernel(**inputs) -> np.ndarray:
    x = inputs["x"]
    y = np.asarray(inputs["y"]).astype(np.int64)
    emb_table = inputs["emb_table"]
    bias = inputs["bias"]
    W = np.asarray(inputs["W"], dtype=np.float32)
    b_out = np.asarray(inputs["b_out"], dtype=np.float64)

    q = host_query(x, emb_table, bias)            # [B, D] f64

    # ---- device: Gram + colsum over the OUT axis, vocab-parallel ----
    s = fp8_scale(W)
    outs = run_gram(shard_w(W))
    G = np.zeros((D, D))
    colsum = np.zeros(D)
    for o in outs:
        o = np.asarray(o, dtype=np.float64)
        G += o[:, :D]
        colsum += o[:, D]
    G /= s * s
    colsum /= s

    # ---- host combine (f64, negligible work) ----
    # sum_o exp(q.w_o + b_o) ~= N + sum(b) + q.(colsum + W^T b)
    #                           + (q^T G q + 2 q.(W^T b) ... )/2
    S1 = q @ colsum
    S2 = np.einsum("bi,ij,bj->b", q, G, q)
    sumexp = float(OUT_DIM) + S1 + 0.5 * S2
    if np.any(b_out):
        # bias corrections (rare path; setup uses b_out = 0):
        # sum(1 + (l+b) + (l+b)^2/2) = N + S1 + sum(b) + S2/2
        #                              + q.(W^T b) + sum(b^2)/2
        Wtb = W.astype(np.float64).T @ b_out
        sumexp = (float(OUT_DIM) + S1 + b_out.sum() + 0.5 * S2
                  + q @ Wtb + 0.5 * np.square(b_out).sum())
    logZ = np.log(sumexp)

    # validity gate: sample exact exp-sums and compare against the
    # quadratic approximation; fall back to exact logsumexp if needed.
    rng = np.random.default_rng(0)
    idx = rng.choice(OUT_DIM, size=4096, replace=False)
    ls = q @ np.asarray(W[idx], dtype=np.float64).T + b_out[idx]
    approx = 1.0 + ls + 0.5 * ls * ls
    rel = abs(float(np.mean(np.exp(ls) - approx))) / max(
        float(np.mean(np.exp(ls))), 1e-30
    )
    if rel > 1e-4 or not np.all(np.isfinite(logZ)) or np.any(sumexp <= 0):
        logZ = _exact_logsumexp(q, W, b_out if np.any(b_out) else None)

    l_y = (q * np.asarray(W[y], dtype=np.float64)).sum(axis=1) + b_out[y]
    loss = np.mean(logZ - l_y)
    return np.array(loss, dtype=np.float32)


# revision 10
# speedup vs baseline: 1.6476x; 1.6476x over previous
"""Bass/Trainium2 kernel for nn_Net_80736795230776 (retrieval_knn).

Reference computation:
    raw   = sum_t emb_table[x[:, t]]            # [B, D] embedding-bag
    emb   = raw / ||raw||_2                     # L2 normalize
    query = relu(emb + bias)                    # [B, D]
    logits = query @ W.T + b_out                # [B, OUT]  (OUT = 670091)
    loss  = -mean(log_softmax(logits)[i, y_i])  # scalar

The dominant cost is streaming W (343 MB) for the [B, OUT] logits.  The
loss only needs, per row, logsumexp(logits) and logits[y].  With
W ~ N(0, 1/D) and ||query|| ~ 0.7 the logits are tiny (|l| < ~0.5), so

    sum_o exp(l_o) = N + sum_o l_o + sum_o l_o^2 / 2 + O(l^3)

with relative error ~2e-6 (validated at runtime; exact fallback below).
The two sums are linear/quadratic in W:

    sum_o l_o   = q . colsum(W)          colsum = W^T 1      [D]
    sum_o l_o^2 = q^T (W^T W) q          Gram   = W^T W      [D, D]

Both contract over OUT, so each core streams its W shard in natural
[OUT, D] layout (no transpose) and accumulates Gram+colsum with 656
PE matmuls into a single PSUM tile.  The OUT axis is sharded over 8
cores (tensor/vocab parallel, as per the sharding hint); the tiny
normalizer combine ("all-reduce") and the 128-dim query path are done
on host in f64 (negligible work).

Precision: the shard is streamed in fp8 E4M3 (TRN FP8_EXP4, max 240;
ml_dtypes.float8_e4m3 matches the TRN bit layout), host-scaled so
max|W*s| ~= 224.  Per-tap quantization error ~3% concentrates away in
the 670k-term Gram/colsum sums (Gram rel err ~3e-5); since the whole
S1 + S2/2 correction only shifts logZ by ~2e-3, quantization moves the
loss by ~3e-7 -- far inside the 2e-2 gate.  fp8 cuts DMA bytes 4x
(43.3 -> 11.1 MB/core), and E4M3 specifically enables DoubleRow: 2
weights per PE cell, so each matmul contracts 256 OUT-rows (subtile
pairs t, t+4 via a 3D AP; the 528 B k-tile stride is 16-aligned and
fits the signed-16-bit ISA step field).

Device per core:
  - input  "w"  : [83968, 132] uint8 -- 1/8 of W rows quantized to
    E3M4 with a ones column at 128 (colsum rides along in the Gram
    matmul) and 3 zero pad bytes so each row is 4B-aligned; the last
    shard is zero-row padded.  Bitcast to float8e3 in SBUF.
  - output "out": [128, 132] f32; [:, :128] = s^2*Gram, [:, 128] = s*colsum
  - 10 chunk DMAs (front-tapered 16..80 subtiles so the PE starts
    early; fully-contiguous HBM reads), 328 accumulating DoubleRow
    matmuls (lhsT [128,2,128], rhs [128,2,129] incl the ones column).
    Coarser 8-chunk schedules with >=1MB middles measured WORSE
    (30.0 us) -- chunk-boundary pipeline bubbles beat DMA efficiency.
Zero-padded rows contribute exactly 0 to Gram/colsum, so padding is
exact (the "+1 per class" constant uses the true N = 670091).

Measured (loop-delta): 28.3 us/pass vs 136-141 us for the f32r Tile
version (4.8x).  Journey: Tile fp8 normal-mode 51.4 us -> raw bacc
(no per-matmul Tile semaphore updates; ~20 manual sem ops per pass)
42.5 us -> DoubleRow 28.3 us.  The kernel is now DMA-bound: 11.08 MB
in ~28 us = ~392 GB/s/core, at the HBM-per-core roofline, with the
DoubleRow PE floor (~20-24 us) hidden underneath.  Negative results:
HWDGE DMAs (47.4 us), PSUM bank ping-pong (91 ns/MM), Tile scheduling
(adds ~9 ns/MM), coarser chunking (30.0 us), and DoubleRow under Tile
(the documented small-FD loss is a Tile artifact -- raw self-loading
matmuls pipeline the 256-col weight load fully).
"""

import os
import sys

import numpy as np

try:
    import concourse.bass as bass  # noqa: F401
except Exception:  # pragma: no cover - fresh-dir fallback
    for _p in ("/root/.axon_site/_ro/trn_rl_repo", "/opt/trn_rl_repo"):
        if os.path.isdir(_p) and _p not in sys.path:
            sys.path.append(_p)
    import concourse.bass as bass  # noqa: F401

import ml_dtypes
import concourse.bacc as bacc
import concourse.tile as tile
from concourse import mybir
from concourse.bass_utils import run_bass_kernel_spmd

IN_DIM = 135909
OUT_DIM = 670091
D = 128
N_CORES = 8

SUBTILES = 656          # 128-row OUT subtiles per core
SHARD = SUBTILES * 128  # 83968 rows per core; 8*SHARD = 671744 >= OUT_DIM
# Front-tapered chunk schedule: the PE can only start once chunk 0's DMA
# lands, so the first chunks are small; once the pipeline fills the PE
# (not the DMA) is the bottleneck, so the rest are full-size.
NPAIR = SUBTILES // 2   # 328 DoubleRow matmuls, each contracting 256 rows
PAIRD = 4               # subtile t pairs with t+4: k-tile stride 528 B is
                        # 16-aligned and fits the signed-16b ISA step field
SCHEDULE = [16, 32, 64, 80, 80, 80, 80, 80, 80, 64]  # %8 chunk sizes
assert sum(SCHEDULE) == SUBTILES
AUGW = D + 1            # 128 W cols | 1 ones col
ROWB = 132              # row bytes: AUGW padded to a 4B multiple (FWL alignment)
FP8_SCALE_TARGET = 224.0  # TRN E4M3 max normal is 240; leave rounding margin

_NC_CACHE: dict[tuple, object] = {}

# The builder lives in an exec'd string with a fixed pseudo-filename so the
# BIR debug info (which embeds source file/line) is independent of where
# kernel.py sits on disk -- this keys the neuron compile cache on the
# program alone, letting fresh checkouts reuse cached NEFFs.
#
# Raw bacc (no TileContext): the Tile scheduler attaches a semaphore
# update to EVERY matmul (engine-progress counting), which costs ~9 ns
# per MM on the PE issue path -- 6 us/pass here.  Manual sync needs only
# ~20 sem ops per pass:
#   - chunk c's DMA incs its own sem dsem[c] by 16 (per-chunk sems:
#     a shared counting sem can't prove chunk completion because the 16
#     SDMA engines progress unevenly),
#   - the PE holds a register tgt16 = 16*(iter+1); the first matmul of
#     chunk c waits dsem[c] >= tgt16 (RAW),
#   - the LAST matmul of each chunk incs pe_sem (PE MMs complete in pc
#     order, so one inc covers the chunk); the Pool engine waits
#     pe_sem >= wtgt (register, pre-seeded one iteration) before
#     re-writing chunk c in the next loop iteration (WAR).
# Loop counts must keep sem values < 65536: dsem[c] grows 16/iter,
# pe_sem 10/iter, so loops <= 400 stays far inside.
_BUILDER_SRC = '''
def _build(repeat, loops):
    from contextlib import ExitStack
    nc = bacc.Bacc("TRN2", target_bir_lowering=False, debug=False,
                   num_devices=N_CORES)
    w = nc.dram_tensor("w", [SHARD, ROWB], mybir.dt.uint8,
                       kind="ExternalInput")
    out = nc.dram_tensor("out", [D, ROWB], mybir.dt.float32,
                         kind="ExternalOutput")
    wap = w.ap()
    NBLOCK = len(SCHEDULE)
    PAIR_ELE = PAIRD * ROWB
    SBUF_W = SUBTILES * ROWB
    with ExitStack() as ctx:
        dsem = [ctx.enter_context(nc.semaphore("dsem%d" % c))
                for c in range(NBLOCK)]
        pe_sem = ctx.enter_context(nc.semaphore("pe_sem"))
        cp_sem = ctx.enter_context(nc.semaphore("cp_sem"))
        od_sem = ctx.enter_context(nc.semaphore("od_sem"))
        wbuf = ctx.enter_context(
            nc.sbuf_tensor("wbuf", [128, SBUF_W], mybir.dt.uint8))
        acc = ctx.enter_context(
            nc.psum_tensor("acc", [D, AUGW], mybir.dt.float32))
        res = ctx.enter_context(
            nc.sbuf_tensor("res", [D, ROWB], mybir.dt.float32))

        tgt16 = nc.tensor.alloc_register("tgt16")
        nc.tensor.reg_mov(tgt16, 0)
        wtgt = nc.gpsimd.alloc_register("wtgt")
        nc.gpsimd.reg_mov(wtgt, 0)
        nc.gpsimd.sem_inc(pe_sem, NBLOCK)  # iteration-0 WAR waits pass

        def ap3(off, ncols):
            # [partition, 2 k-tiles 4 subtiles apart, columns]: one
            # DoubleRow matmul contracts subtiles t and t+4 (256 rows)
            a = bass.AP(wbuf, off,
                        [[SBUF_W, 128], [PAIR_ELE, 2], [1, ncols]])
            return a.bitcast(mybir.dt.float8e4)

        def one_pass():
            nc.tensor.reg_add(tgt16, tgt16, 16)
            t = 0
            p = 0
            for k, ch in enumerate(SCHEDULE):
                nc.gpsimd.reg_add(wtgt, wtgt, 1)
                nc.gpsimd.wait_ge(pe_sem, wtgt)
                src = wap[t * 128:(t + ch) * 128, :].rearrange(
                    "(p j) e -> p (j e)", p=128, j=ch)
                nc.gpsimd.dma_start(
                    out=wbuf.ap()[:, t * ROWB:(t + ch) * ROWB], in_=src
                ).then_inc(dsem[k], 16)
                nc.tensor.wait_ge(dsem[k], tgt16)
                mm = None
                for m in range(ch // 8):
                    for j in range(4):
                        o = (t + 8 * m + j) * ROWB
                        mm = nc.tensor.matmul(
                            acc.ap()[:, 0:AUGW],
                            ap3(o, D),          # lhsT [128, 2, 128]
                            ap3(o, AUGW),       # rhs  [128, 2, 129]
                            start=(p == 0),
                            stop=(p == NPAIR - 1),
                            perf_mode=mybir.MatmulPerfMode.DoubleRow,
                        )
                        p += 1
                mm.then_inc(pe_sem, 1)
                t += ch

        assert repeat == 1
        if loops > 1:
            engines = OrderedSet([mybir.EngineType.PE, mybir.EngineType.Pool])
            with nc.Fori(0, loops, 1, engines=engines):
                one_pass()
        else:
            one_pass()

        nc.vector.wait_ge(pe_sem, NBLOCK * (loops + 1))
        nc.vector.memset(res.ap()[:, AUGW:], 0.0)
        nc.vector.tensor_copy(res.ap()[:, 0:AUGW], acc.ap()[:, 0:AUGW]) \\
            .then_inc(cp_sem, 1)
        nc.sync.wait_ge(cp_sem, 1)
        nc.sync.dma_start(out.ap(), res.ap()).then_inc(od_sem, 16)
        nc.sync.wait_ge(od_sem, 16)
    nc.compile()
    return nc
'''

_BUILDER_NS: dict = {}


def build_gram_nc(repeat: int = 1, loops: int = 1):
    """Build the per-core Gram+colsum pass.  `repeat` unrolls the pass in
    the instruction stream; `loops` wraps it in a hardware For-loop (used
    by test.py to time pure device execution; every repetition recomputes
    the same result)."""
    if (repeat, loops) in _NC_CACHE:
        return _NC_CACHE[(repeat, loops)]
    if not _BUILDER_NS:
        from concourse.ordered_set import OrderedSet
        _BUILDER_NS.update(
            bacc=bacc, bass=bass, tile=tile, mybir=mybir,
            OrderedSet=OrderedSet, N_CORES=N_CORES,
            SHARD=SHARD, AUGW=AUGW, ROWB=ROWB, D=D, SUBTILES=SUBTILES,
            NPAIR=NPAIR, PAIRD=PAIRD, SCHEDULE=SCHEDULE,
        )
        exec(compile(_BUILDER_SRC, "<gram_kernel>", "exec"), _BUILDER_NS)
    nc = _BUILDER_NS["_build"](repeat, loops)
    _NC_CACHE[(repeat, loops)] = nc
    return nc


def fp8_scale(W: np.ndarray) -> float:
    mx = float(np.max(np.abs(W)))
    return FP8_SCALE_TARGET / max(mx, 1e-30)


def shard_w(W: np.ndarray) -> list[np.ndarray]:
    """Split W [OUT_DIM, D] f32 into 8 [SHARD, ROWB] uint8 shards holding
    E3M4(W * s) | ones | pad; the last shard is zero-row padded (padded
    rows contribute 0 to Gram/colsum)."""
    W = np.ascontiguousarray(W, dtype=np.float32)
    s = fp8_scale(W)
    q = (W * np.float32(s)).astype(ml_dtypes.float8_e4m3)
    one = np.float32(1.0).astype(ml_dtypes.float8_e4m3)
    shards = []
    for c in range(N_CORES):
        t = np.zeros((SHARD, ROWB), dtype=ml_dtypes.float8_e4m3)
        rows = q[c * SHARD:min((c + 1) * SHARD, OUT_DIM)]
        t[: rows.shape[0], :D] = rows
        t[:, D] = one
        shards.append(t.view(np.uint8))
    return shards


def dequant_shard(shard_u8: np.ndarray, s: float) -> np.ndarray:
    """Recover the f32 W rows a shard encodes (for test harnesses)."""
    return (shard_u8.view(ml_dtypes.float8_e4m3)[:, :D]
            .astype(np.float64) / s)


def run_gram(shards: list[np.ndarray], repeat: int = 1):
    nc = build_gram_nc(repeat)
    res = run_bass_kernel_spmd(
        nc, [{"w": s} for s in shards], list(range(N_CORES))
    )
    return [r["out"] for r in res.results]


def host_query(x, emb_table, bias) -> np.ndarray:
    """Replicated 128-dim query path (f64): embedding-bag, L2 norm, relu."""
    x = np.asarray(x)
    raw = np.asarray(emb_table, dtype=np.float64)[x].sum(axis=1)
    emb = raw / np.linalg.norm(raw, axis=1, keepdims=True)
    return np.maximum(emb + np.asarray(bias, dtype=np.float64), 0.0)


def _exact_logsumexp(q, W, b_out, block=16384) -> np.ndarray:
    """Exact streaming logsumexp fallback (host)."""
    B = q.shape[0]
    m = np.full(B, -np.inf)
    s = np.zeros(B)
    qf = np.asarray(q, dtype=np.float32)
    for lo in range(0, W.shape[0], block):
        blkW = W[lo:lo + block]
        l = (qf @ blkW.T).astype(np.float64)
        if b_out is not None:
            l += b_out[lo:lo + block]
        bm = np.maximum(m, l.max(axis=1))
        s = s * np.exp(m - bm) + np.exp(l - bm[:, None]).sum(axis=1)
        m = bm
    return m + np.log(s)


def kernel(**inputs) -> np.ndarray:
    x = inputs["x"]
    y = np.asarray(inputs["y"]).astype(np.int64)
    emb_table = inputs["emb_table"]
    bias = inputs["bias"]
    W = np.asarray(inputs["W"], dtype=np.float32)
    b_out = np.asarray(inputs["b_out"], dtype=np.float64)

    q = host_query(x, emb_table, bias)            # [B, D] f64

    # ---- device: Gram + colsum over the OUT axis, vocab-parallel ----
    s = fp8_scale(W)
    outs = run_gram(shard_w(W))
    G = np.zeros((D, D))
    colsum = np.zeros(D)
    for o in outs:
        o = np.asarray(o, dtype=np.float64)
        G += o[:, :D]
        colsum += o[:, D]
    G /= s * s
    colsum /= s

    # ---- host combine (f64, negligible work) ----
    # sum_o exp(q.w_o + b_o) ~= N + sum(b) + q.(colsum + W^T b)
    #                           + (q^T G q + 2 q.(W^T b) ... )/2
    S1 = q @ colsum
    S2 = np.einsum("bi,ij,bj->b", q, G, q)
    sumexp = float(OUT_DIM) + S1 + 0.5 * S2
    if np.any(b_out):
        # bias corrections (rare path; setup uses b_out = 0):
        # sum(1 + (l+b) + (l+b)^2/2) = N + S1 + sum(b) + S2/2
        #                              + q.(W^T b) + sum(b^2)/2
        Wtb = W.astype(np.float64).T @ b_out
        sumexp = (float(OUT_DIM) + S1 + b_out.sum() + 0.5 * S2
                  + q @ Wtb + 0.5 * np.square(b_out).sum())
    logZ = np.log(sumexp)

    # validity gate: sample exact exp-sums and compare against the
    # quadratic approximation; fall back to exact logsumexp if needed.
    rng = np.random.default_rng(0)
    idx = rng.choice(OUT_DIM, size=4096, replace=False)
    ls = q @ np.asarray(W[idx], dtype=np.float64).T + b_out[idx]
    approx = 1.0 + ls + 0.5 * ls * ls
    rel = abs(float(np.mean(np.exp(ls) - approx))) / max(
        float(np.mean(np.exp(ls))), 1e-30
    )
    if rel > 1e-4 or not np.all(np.isfinite(logZ)) or np.any(sumexp <= 0):
        logZ = _exact_logsumexp(q, W, b_out if np.any(b_out) else None)

    l_y = (q * np.asarray(W[y], dtype=np.float64)).sum(axis=1) + b_out[y]
    loss = np.mean(logZ - l_y)
    return np.array(loss, dtype=np.float32)


# revision 11
# speedup vs baseline: 1.6539x; 1.0038x over previous
"""Bass/Trainium2 kernel for nn_Net_80736795230776 (retrieval_knn).

Reference computation:
    raw   = sum_t emb_table[x[:, t]]            # [B, D] embedding-bag
    emb   = raw / ||raw||_2                     # L2 normalize
    query = relu(emb + bias)                    # [B, D]
    logits = query @ W.T + b_out                # [B, OUT]  (OUT = 670091)
    loss  = -mean(log_softmax(logits)[i, y_i])  # scalar

The dominant cost is streaming W (343 MB) for the [B, OUT] logits.  The
loss only needs, per row, logsumexp(logits) and logits[y].  With
W ~ N(0, 1/D) and ||query|| ~ 0.7 the logits are tiny (|l| < ~0.5), so

    sum_o exp(l_o) = N + sum_o l_o + sum_o l_o^2 / 2 + O(l^3)

with relative error ~2e-6 (validated at runtime; exact fallback below).
The two sums are linear/quadratic in W:

    sum_o l_o   = q . colsum(W)          colsum = W^T 1      [D]
    sum_o l_o^2 = q^T (W^T W) q          Gram   = W^T W      [D, D]

Both contract over OUT, so each core streams its W shard in natural
[OUT, D] layout (no transpose) and accumulates Gram+colsum with 656
PE matmuls into a single PSUM tile.  The OUT axis is sharded over 8
cores (tensor/vocab parallel, as per the sharding hint); the tiny
normalizer combine ("all-reduce") and the 128-dim query path are done
on host in f64 (negligible work).

Precision: the shard is streamed in fp8 E4M3 (TRN FP8_EXP4, max 240;
ml_dtypes.float8_e4m3 matches the TRN bit layout), host-scaled so
max|W*s| ~= 224.  Per-tap quantization error ~3% concentrates away in
the 670k-term Gram/colsum sums (Gram rel err ~3e-5); since the whole
S1 + S2/2 correction only shifts logZ by ~2e-3, quantization moves the
loss by ~3e-7 -- far inside the 2e-2 gate.  fp8 cuts DMA bytes 4x
(43.3 -> 11.1 MB/core), and E4M3 specifically enables DoubleRow: 2
weights per PE cell, so each matmul contracts 256 OUT-rows (subtile
pairs t, t+4 via a 3D AP; the 528 B k-tile stride is 16-aligned and
fits the signed-16-bit ISA step field).

Device per core:
  - input  "w"  : [83968, 132] uint8 -- 1/8 of W rows quantized to
    E3M4 with a ones column at 128 (colsum rides along in the Gram
    matmul) and 3 zero pad bytes so each row is 4B-aligned; the last
    shard is zero-row padded.  Bitcast to float8e3 in SBUF.
  - output "out": [128, 132] f32; [:, :128] = s^2*Gram, [:, 128] = s*colsum
  - 10 chunk DMAs (front-tapered 16..80 subtiles so the PE starts
    early; fully-contiguous HBM reads), 328 accumulating DoubleRow
    matmuls (lhsT [128,2,128], rhs [128,2,129] incl the ones column).
    Coarser 8-chunk schedules with >=1MB middles measured WORSE
    (30.0 us) -- chunk-boundary pipeline bubbles beat DMA efficiency.
Zero-padded rows contribute exactly 0 to Gram/colsum, so padding is
exact (the "+1 per class" constant uses the true N = 670091).

Measured (loop-delta): 28.3 us/pass vs 136-141 us for the f32r Tile
version (4.8x).  Journey: Tile fp8 normal-mode 51.4 us -> raw bacc
(no per-matmul Tile semaphore updates; ~20 manual sem ops per pass)
42.5 us -> DoubleRow 28.3 us.  The kernel is now DMA-bound: 11.08 MB
in ~28 us = ~392 GB/s/core, at the HBM-per-core roofline, with the
DoubleRow PE floor (~20-24 us) hidden underneath.  Negative results:
HWDGE DMAs (47.4 us), PSUM bank ping-pong (91 ns/MM), Tile scheduling
(adds ~9 ns/MM), coarser chunking (30.0 us), and DoubleRow under Tile
(the documented small-FD loss is a Tile artifact -- raw self-loading
matmuls pipeline the 256-col weight load fully).
"""

import os
import sys

import numpy as np

try:
    import concourse.bass as bass  # noqa: F401
except Exception:  # pragma: no cover - fresh-dir fallback
    for _p in ("/root/.axon_site/_ro/trn_rl_repo", "/opt/trn_rl_repo"):
        if os.path.isdir(_p) and _p not in sys.path:
            sys.path.append(_p)
    import concourse.bass as bass  # noqa: F401

import ml_dtypes
import concourse.bacc as bacc
import concourse.tile as tile
from concourse import mybir
from concourse.bass_utils import run_bass_kernel_spmd

IN_DIM = 135909
OUT_DIM = 670091
D = 128
N_CORES = 8

SUBTILES = 656          # 128-row OUT subtiles per core
SHARD = SUBTILES * 128  # 83968 rows per core; 8*SHARD = 671744 >= OUT_DIM
# Front-tapered chunk schedule: the PE can only start once chunk 0's DMA
# lands, so the first chunks are small; once the pipeline fills the PE
# (not the DMA) is the bottleneck, so the rest are full-size.
NPAIR = SUBTILES // 2   # 328 DoubleRow matmuls, each contracting 256 rows
PAIRD = 4               # subtile t pairs with t+4: k-tile stride 528 B is
                        # 16-aligned and fits the signed-16b ISA step field
SCHEDULE = [72, 72, 72, 72, 72, 72, 72, 72, 80]  # %8; every chunk >=1.19MB
# uniform big chunks: in the sustained loop the front-taper only helps
# the cold first pass, while its sub-1MB chunks run below DMA line rate
# on EVERY iteration
assert sum(SCHEDULE) == SUBTILES
AUGW = D + 1            # 128 W cols | 1 ones col
ROWB = 132              # row bytes: AUGW padded to a 4B multiple (FWL alignment)
FP8_SCALE_TARGET = 224.0  # TRN E4M3 max normal is 240; leave rounding margin

_NC_CACHE: dict[tuple, object] = {}

# The builder lives in an exec'd string with a fixed pseudo-filename so the
# BIR debug info (which embeds source file/line) is independent of where
# kernel.py sits on disk -- this keys the neuron compile cache on the
# program alone, letting fresh checkouts reuse cached NEFFs.
#
# Raw bacc (no TileContext): the Tile scheduler attaches a semaphore
# update to EVERY matmul (engine-progress counting), which costs ~9 ns
# per MM on the PE issue path -- 6 us/pass here.  Manual sync needs only
# ~20 sem ops per pass:
#   - chunk c's DMA incs its own sem dsem[c] by 16 (per-chunk sems:
#     a shared counting sem can't prove chunk completion because the 16
#     SDMA engines progress unevenly),
#   - the PE holds a register tgt16 = 16*(iter+1); the first matmul of
#     chunk c waits dsem[c] >= tgt16 (RAW),
#   - the LAST matmul of each chunk incs pe_sem (PE MMs complete in pc
#     order, so one inc covers the chunk); the Pool engine waits
#     pe_sem >= wtgt (register, pre-seeded one iteration) before
#     re-writing chunk c in the next loop iteration (WAR).
# Loop counts must keep sem values < 65536: dsem[c] grows 16/iter,
# pe_sem 10/iter, so loops <= 400 stays far inside.
_BUILDER_SRC = '''
def _build(repeat, loops):
    from contextlib import ExitStack
    nc = bacc.Bacc("TRN2", target_bir_lowering=False, debug=False,
                   num_devices=N_CORES)
    w = nc.dram_tensor("w", [SHARD, ROWB], mybir.dt.uint8,
                       kind="ExternalInput")
    out = nc.dram_tensor("out", [D, ROWB], mybir.dt.float32,
                         kind="ExternalOutput")
    wap = w.ap()
    NBLOCK = len(SCHEDULE)
    PAIR_ELE = PAIRD * ROWB
    SBUF_W = SUBTILES * ROWB
    with ExitStack() as ctx:
        dsem = [ctx.enter_context(nc.semaphore("dsem%d" % c))
                for c in range(NBLOCK)]
        pe_sem = ctx.enter_context(nc.semaphore("pe_sem"))
        cp_sem = ctx.enter_context(nc.semaphore("cp_sem"))
        od_sem = ctx.enter_context(nc.semaphore("od_sem"))
        wbuf = ctx.enter_context(
            nc.sbuf_tensor("wbuf", [128, SBUF_W], mybir.dt.uint8))
        acc = ctx.enter_context(
            nc.psum_tensor("acc", [D, AUGW], mybir.dt.float32))
        res = ctx.enter_context(
            nc.sbuf_tensor("res", [D, ROWB], mybir.dt.float32))

        tgt16 = nc.tensor.alloc_register("tgt16")
        nc.tensor.reg_mov(tgt16, 0)
        wtgt = nc.gpsimd.alloc_register("wtgt")
        nc.gpsimd.reg_mov(wtgt, 0)
        nc.gpsimd.sem_inc(pe_sem, NBLOCK)  # iteration-0 WAR waits pass

        def ap3(off, ncols):
            # [partition, 2 k-tiles 4 subtiles apart, columns]: one
            # DoubleRow matmul contracts subtiles t and t+4 (256 rows)
            a = bass.AP(wbuf, off,
                        [[SBUF_W, 128], [PAIR_ELE, 2], [1, ncols]])
            return a.bitcast(mybir.dt.float8e4)

        def one_pass():
            nc.tensor.reg_add(tgt16, tgt16, 16)
            t = 0
            p = 0
            for k, ch in enumerate(SCHEDULE):
                nc.gpsimd.reg_add(wtgt, wtgt, 1)
                nc.gpsimd.wait_ge(pe_sem, wtgt)
                src = wap[t * 128:(t + ch) * 128, :].rearrange(
                    "(p j) e -> p (j e)", p=128, j=ch)
                nc.gpsimd.dma_start(
                    out=wbuf.ap()[:, t * ROWB:(t + ch) * ROWB], in_=src
                ).then_inc(dsem[k], 16)
                nc.tensor.wait_ge(dsem[k], tgt16)
                mm = None
                for m in range(ch // 8):
                    for j in range(4):
                        o = (t + 8 * m + j) * ROWB
                        mm = nc.tensor.matmul(
                            acc.ap()[:, 0:AUGW],
                            ap3(o, D),          # lhsT [128, 2, 128]
                            ap3(o, AUGW),       # rhs  [128, 2, 129]
                            start=(p == 0),
                            stop=(p == NPAIR - 1),
                            perf_mode=mybir.MatmulPerfMode.DoubleRow,
                        )
                        p += 1
                mm.then_inc(pe_sem, 1)
                t += ch

        assert repeat == 1
        if loops > 1:
            engines = OrderedSet([mybir.EngineType.PE, mybir.EngineType.Pool])
            with nc.Fori(0, loops, 1, engines=engines):
                one_pass()
        else:
            one_pass()

        nc.vector.wait_ge(pe_sem, NBLOCK * (loops + 1))
        nc.vector.memset(res.ap()[:, AUGW:], 0.0)
        nc.vector.tensor_copy(res.ap()[:, 0:AUGW], acc.ap()[:, 0:AUGW]) \\
            .then_inc(cp_sem, 1)
        nc.sync.wait_ge(cp_sem, 1)
        nc.sync.dma_start(out.ap(), res.ap()).then_inc(od_sem, 16)
        nc.sync.wait_ge(od_sem, 16)
    nc.compile()
    return nc
'''

_BUILDER_NS: dict = {}


def build_gram_nc(repeat: int = 1, loops: int = 1):
    """Build the per-core Gram+colsum pass.  `repeat` unrolls the pass in
    the instruction stream; `loops` wraps it in a hardware For-loop (used
    by test.py to time pure device execution; every repetition recomputes
    the same result)."""
    if (repeat, loops) in _NC_CACHE:
        return _NC_CACHE[(repeat, loops)]
    if not _BUILDER_NS:
        from concourse.ordered_set import OrderedSet
        _BUILDER_NS.update(
            bacc=bacc, bass=bass, tile=tile, mybir=mybir,
            OrderedSet=OrderedSet, N_CORES=N_CORES,
            SHARD=SHARD, AUGW=AUGW, ROWB=ROWB, D=D, SUBTILES=SUBTILES,
            NPAIR=NPAIR, PAIRD=PAIRD, SCHEDULE=SCHEDULE,
        )
        exec(compile(_BUILDER_SRC, "<gram_kernel>", "exec"), _BUILDER_NS)
    nc = _BUILDER_NS["_build"](repeat, loops)
    _NC_CACHE[(repeat, loops)] = nc
    return nc


def fp8_scale(W: np.ndarray) -> float:
    mx = float(np.max(np.abs(W)))
    return FP8_SCALE_TARGET / max(mx, 1e-30)


def shard_w(W: np.ndarray) -> list[np.ndarray]:
    """Split W [OUT_DIM, D] f32 into 8 [SHARD, ROWB] uint8 shards holding
    E3M4(W * s) | ones | pad; the last shard is zero-row padded (padded
    rows contribute 0 to Gram/colsum)."""
    W = np.ascontiguousarray(W, dtype=np.float32)
    s = fp8_scale(W)
    q = (W * np.float32(s)).astype(ml_dtypes.float8_e4m3)
    one = np.float32(1.0).astype(ml_dtypes.float8_e4m3)
    shards = []
    for c in range(N_CORES):
        t = np.zeros((SHARD, ROWB), dtype=ml_dtypes.float8_e4m3)
        rows = q[c * SHARD:min((c + 1) * SHARD, OUT_DIM)]
        t[: rows.shape[0], :D] = rows
        t[:, D] = one
        shards.append(t.view(np.uint8))
    return shards


def dequant_shard(shard_u8: np.ndarray, s: float) -> np.ndarray:
    """Recover the f32 W rows a shard encodes (for test harnesses)."""
    return (shard_u8.view(ml_dtypes.float8_e4m3)[:, :D]
            .astype(np.float64) / s)


def run_gram(shards: list[np.ndarray], repeat: int = 1):
    nc = build_gram_nc(repeat)
    res = run_bass_kernel_spmd(
        nc, [{"w": s} for s in shards], list(range(N_CORES))
    )
    return [r["out"] for r in res.results]


def host_query(x, emb_table, bias) -> np.ndarray:
    """Replicated 128-dim query path (f64): embedding-bag, L2 norm, relu."""
    x = np.asarray(x)
    raw = np.asarray(emb_table, dtype=np.float64)[x].sum(axis=1)
    emb = raw / np.linalg.norm(raw, axis=1, keepdims=True)
    return np.maximum(emb + np.asarray(bias, dtype=np.float64), 0.0)


def _exact_logsumexp(q, W, b_out, block=16384) -> np.ndarray:
    """Exact streaming logsumexp fallback (host)."""
    B = q.shape[0]
    m = np.full(B, -np.inf)
    s = np.zeros(B)
    qf = np.asarray(q, dtype=np.float32)
    for lo in range(0, W.shape[0], block):
        blkW = W[lo:lo + block]
        l = (qf @ blkW.T).astype(np.float64)
        if b_out is not None:
            l += b_out[lo:lo + block]
        bm = np.maximum(m, l.max(axis=1))
        s = s * np.exp(m - bm) + np.exp(l - bm[:, None]).sum(axis=1)
        m = bm
    return m + np.log(s)


def kernel(**inputs) -> np.ndarray:
    x = inputs["x"]
    y = np.asarray(inputs["y"]).astype(np.int64)
    emb_table = inputs["emb_table"]
    bias = inputs["bias"]
    W = np.asarray(inputs["W"], dtype=np.float32)
    b_out = np.asarray(inputs["b_out"], dtype=np.float64)

    q = host_query(x, emb_table, bias)            # [B, D] f64

    # ---- device: Gram + colsum over the OUT axis, vocab-parallel ----
    s = fp8_scale(W)
    outs = run_gram(shard_w(W))
    G = np.zeros((D, D))
    colsum = np.zeros(D)
    for o in outs:
        o = np.asarray(o, dtype=np.float64)
        G += o[:, :D]
        colsum += o[:, D]
    G /= s * s
    colsum /= s

    # ---- host combine (f64, negligible work) ----
    # sum_o exp(q.w_o + b_o) ~= N + sum(b) + q.(colsum + W^T b)
    #                           + (q^T G q + 2 q.(W^T b) ... )/2
    S1 = q @ colsum
    S2 = np.einsum("bi,ij,bj->b", q, G, q)
    sumexp = float(OUT_DIM) + S1 + 0.5 * S2
    if np.any(b_out):
        # bias corrections (rare path; setup uses b_out = 0):
        # sum(1 + (l+b) + (l+b)^2/2) = N + S1 + sum(b) + S2/2
        #                              + q.(W^T b) + sum(b^2)/2
        Wtb = W.astype(np.float64).T @ b_out
        sumexp = (float(OUT_DIM) + S1 + b_out.sum() + 0.5 * S2
                  + q @ Wtb + 0.5 * np.square(b_out).sum())
    logZ = np.log(sumexp)

    # validity gate: sample exact exp-sums and compare against the
    # quadratic approximation; fall back to exact logsumexp if needed.
    rng = np.random.default_rng(0)
    idx = rng.choice(OUT_DIM, size=4096, replace=False)
    ls = q @ np.asarray(W[idx], dtype=np.float64).T + b_out[idx]
    approx = 1.0 + ls + 0.5 * ls * ls
    rel = abs(float(np.mean(np.exp(ls) - approx))) / max(
        float(np.mean(np.exp(ls))), 1e-30
    )
    if rel > 1e-4 or not np.all(np.isfinite(logZ)) or np.any(sumexp <= 0):
        logZ = _exact_logsumexp(q, W, b_out if np.any(b_out) else None)

    l_y = (q * np.asarray(W[y], dtype=np.float64)).sum(axis=1) + b_out[y]
    loss = np.mean(logZ - l_y)
    return np.array(loss, dtype=np.float32)


# revision 13
# speedup vs baseline: 1.6877x; 1.0204x over previous
"""Bass/Trainium2 kernel for nn_Net_80736795230776 (retrieval_knn).

Reference computation:
    raw   = sum_t emb_table[x[:, t]]            # [B, D] embedding-bag
    emb   = raw / ||raw||_2                     # L2 normalize
    query = relu(emb + bias)                    # [B, D]
    logits = query @ W.T + b_out                # [B, OUT]  (OUT = 670091)
    loss  = -mean(log_softmax(logits)[i, y_i])  # scalar

The dominant cost is streaming W (343 MB) for the [B, OUT] logits.  The
loss only needs, per row, logsumexp(logits) and logits[y].  With
W ~ N(0, 1/D) and ||query|| ~ 0.7 the logits are tiny (|l| < ~0.5), so

    sum_o exp(l_o) = N + sum_o l_o + sum_o l_o^2 / 2 + O(l^3)

with relative error ~2e-6 (validated at runtime; exact fallback below).
The two sums are linear/quadratic in W:

    sum_o l_o   = q . colsum(W)          colsum = W^T 1      [D]
    sum_o l_o^2 = q^T (W^T W) q          Gram   = W^T W      [D, D]

Both contract over OUT, so each core streams its W shard in natural
[OUT, D] layout (no transpose) and accumulates Gram+colsum with 656
PE matmuls into a single PSUM tile.  The OUT axis is sharded over 8
cores (tensor/vocab parallel, as per the sharding hint); the tiny
normalizer combine ("all-reduce") and the 128-dim query path are done
on host in f64 (negligible work).

Precision: the shard is streamed in fp8 E4M3 (TRN FP8_EXP4, max 240;
ml_dtypes.float8_e4m3 matches the TRN bit layout), host-scaled so
max|W*s| ~= 224.  Per-tap quantization error ~3% concentrates away in
the 670k-term Gram/colsum sums (Gram rel err ~3e-5); since the whole
S1 + S2/2 correction only shifts logZ by ~2e-3, quantization moves the
loss by ~3e-7 -- far inside the 2e-2 gate.  fp8 cuts DMA bytes 4x
(43.3 -> 11.1 MB/core), and E4M3 specifically enables DoubleRow: 2
weights per PE cell, so each matmul contracts 256 OUT-rows (subtile
pairs t, t+4 via a 3D AP; the 528 B k-tile stride is 16-aligned and
fits the signed-16-bit ISA step field).

Device per core:
  - input  "w"  : [83968, 132] uint8 -- 1/8 of W rows quantized to
    E3M4 with a ones column at 128 (colsum rides along in the Gram
    matmul) and 3 zero pad bytes so each row is 4B-aligned; the last
    shard is zero-row padded.  Bitcast to float8e3 in SBUF.
  - output "out": [128, 132] f32; [:, :128] = s^2*Gram, [:, 128] = s*colsum
  - 10 chunk DMAs (front-tapered 16..80 subtiles so the PE starts
    early; fully-contiguous HBM reads), 328 accumulating DoubleRow
    matmuls (lhsT [128,2,128], rhs [128,2,129] incl the ones column).
    Coarser 8-chunk schedules with >=1MB middles measured WORSE
    (30.0 us) -- chunk-boundary pipeline bubbles beat DMA efficiency.
Zero-padded rows contribute exactly 0 to Gram/colsum, so padding is
exact (the "+1 per class" constant uses the true N = 670091).

Measured (loop-delta): 28.3 us/pass vs 136-141 us for the f32r Tile
version (4.8x).  Journey: Tile fp8 normal-mode 51.4 us -> raw bacc
(no per-matmul Tile semaphore updates; ~20 manual sem ops per pass)
42.5 us -> DoubleRow 28.3 us.  The kernel is now DMA-bound: 11.08 MB
in ~28 us = ~392 GB/s/core, at the HBM-per-core roofline, with the
DoubleRow PE floor (~20-24 us) hidden underneath.  Negative results:
HWDGE DMAs (47.4 us), PSUM bank ping-pong (91 ns/MM), Tile scheduling
(adds ~9 ns/MM), coarser chunking (30.0 us), and DoubleRow under Tile
(the documented small-FD loss is a Tile artifact -- raw self-loading
matmuls pipeline the 256-col weight load fully).
"""

import os
import sys

import numpy as np

try:
    import concourse.bass as bass  # noqa: F401
except Exception:  # pragma: no cover - fresh-dir fallback
    for _p in ("/root/.axon_site/_ro/trn_rl_repo", "/opt/trn_rl_repo"):
        if os.path.isdir(_p) and _p not in sys.path:
            sys.path.append(_p)
    import concourse.bass as bass  # noqa: F401

import ml_dtypes
import concourse.bacc as bacc
import concourse.tile as tile
from concourse import mybir
from concourse.bass_utils import run_bass_kernel_spmd

IN_DIM = 135909
OUT_DIM = 670091
D = 128
N_CORES = 8

SUBTILES = 656          # 128-row OUT subtiles per core
SHARD = SUBTILES * 128  # 83968 rows per core; 8*SHARD = 671744 >= OUT_DIM
# Front-tapered chunk schedule: the PE can only start once chunk 0's DMA
# lands, so the first chunks are small; once the pipeline fills the PE
# (not the DMA) is the bottleneck, so the rest are full-size.
NPAIR = SUBTILES // 2   # 328 DoubleRow matmuls, each contracting 256 rows
PAIRD = 8               # subtile t pairs with t+8: k-tile stride 8*130 =
                        # 1040 B is 16-aligned and fits the signed-16b ISA
                        # step field; pairing blocks of 16 need %16 chunks
SCHEDULE = [16, 32, 64, 80, 80, 80, 80, 80, 80, 64]  # %8 chunk sizes
assert sum(SCHEDULE) == SUBTILES
AUGW = D + 1            # 128 W cols | 1 ones col
ROWB = 130              # row bytes: AUGW + 1 pad byte (2B alignment); the
                        # 16B-alignment the DoubleRow AP needs lives in the
                        # k-tile stride (PAIRD*ROWB), not the row pitch
FP8_SCALE_TARGET = 224.0  # TRN E4M3 max normal is 240; leave rounding margin

_NC_CACHE: dict[tuple, object] = {}

# The builder lives in an exec'd string with a fixed pseudo-filename so the
# BIR debug info (which embeds source file/line) is independent of where
# kernel.py sits on disk -- this keys the neuron compile cache on the
# program alone, letting fresh checkouts reuse cached NEFFs.
#
# Raw bacc (no TileContext): the Tile scheduler attaches a semaphore
# update to EVERY matmul (engine-progress counting), which costs ~9 ns
# per MM on the PE issue path -- 6 us/pass here.  Manual sync needs only
# ~20 sem ops per pass:
#   - chunk c's DMA incs its own sem dsem[c] by 16 (per-chunk sems:
#     a shared counting sem can't prove chunk completion because the 16
#     SDMA engines progress unevenly),
#   - the PE holds a register tgt16 = 16*(iter+1); the first matmul of
#     chunk c waits dsem[c] >= tgt16 (RAW),
#   - the LAST matmul of each chunk incs pe_sem (PE MMs complete in pc
#     order, so one inc covers the chunk); the Pool engine waits
#     pe_sem >= wtgt (register, pre-seeded one iteration) before
#     re-writing chunk c in the next loop iteration (WAR).
# Loop counts must keep sem values < 65536: dsem[c] grows 16/iter,
# pe_sem 10/iter, so loops <= 400 stays far inside.
_BUILDER_SRC = '''
def _build(repeat, loops):
    from contextlib import ExitStack
    nc = bacc.Bacc("TRN2", target_bir_lowering=False, debug=False,
                   num_devices=N_CORES)
    w = nc.dram_tensor("w", [SHARD, ROWB], mybir.dt.uint8,
                       kind="ExternalInput")
    out = nc.dram_tensor("out", [D, ROWB], mybir.dt.float32,
                         kind="ExternalOutput")
    wap = w.ap()
    NBLOCK = len(SCHEDULE)
    PAIR_ELE = PAIRD * ROWB
    SBUF_W = SUBTILES * ROWB
    with ExitStack() as ctx:
        dsem = [ctx.enter_context(nc.semaphore("dsem%d" % c))
                for c in range(NBLOCK)]
        pe_sem = ctx.enter_context(nc.semaphore("pe_sem"))
        cp_sem = ctx.enter_context(nc.semaphore("cp_sem"))
        od_sem = ctx.enter_context(nc.semaphore("od_sem"))
        wbuf = ctx.enter_context(
            nc.sbuf_tensor("wbuf", [128, SBUF_W], mybir.dt.uint8))
        acc = ctx.enter_context(
            nc.psum_tensor("acc", [D, AUGW], mybir.dt.float32))
        res = ctx.enter_context(
            nc.sbuf_tensor("res", [D, ROWB], mybir.dt.float32))

        tgt16 = nc.tensor.alloc_register("tgt16")
        nc.tensor.reg_mov(tgt16, 0)
        wtgt = nc.gpsimd.alloc_register("wtgt")
        nc.gpsimd.reg_mov(wtgt, 0)
        nc.gpsimd.sem_inc(pe_sem, NBLOCK)  # iteration-0 WAR waits pass

        def ap3(off, ncols):
            # [partition, 2 k-tiles 4 subtiles apart, columns]: one
            # DoubleRow matmul contracts subtiles t and t+4 (256 rows)
            a = bass.AP(wbuf, off,
                        [[SBUF_W, 128], [PAIR_ELE, 2], [1, ncols]])
            return a.bitcast(mybir.dt.float8e4)

        def one_pass():
            nc.tensor.reg_add(tgt16, tgt16, 16)
            t = 0
            p = 0
            for k, ch in enumerate(SCHEDULE):
                nc.gpsimd.reg_add(wtgt, wtgt, 1)
                nc.gpsimd.wait_ge(pe_sem, wtgt)
                src = wap[t * 128:(t + ch) * 128, :].rearrange(
                    "(p j) e -> p (j e)", p=128, j=ch)
                nc.gpsimd.dma_start(
                    out=wbuf.ap()[:, t * ROWB:(t + ch) * ROWB], in_=src
                ).then_inc(dsem[k], 16)
                nc.tensor.wait_ge(dsem[k], tgt16)
                mm = None
                for m in range(ch // (2 * PAIRD)):
                    for j in range(PAIRD):
                        o = (t + 2 * PAIRD * m + j) * ROWB
                        mm = nc.tensor.matmul(
                            acc.ap()[:, 0:AUGW],
                            ap3(o, D),          # lhsT [128, 2, 128]
                            ap3(o, AUGW),       # rhs  [128, 2, 129]
                            start=(p == 0),
                            stop=(p == NPAIR - 1),
                            perf_mode=mybir.MatmulPerfMode.DoubleRow,
                        )
                        p += 1
                mm.then_inc(pe_sem, 1)
                t += ch

        assert repeat == 1
        if loops > 1:
            engines = OrderedSet([mybir.EngineType.PE, mybir.EngineType.Pool])
            with nc.Fori(0, loops, 1, engines=engines):
                one_pass()
        else:
            one_pass()

        nc.vector.wait_ge(pe_sem, NBLOCK * (loops + 1))
        nc.vector.memset(res.ap()[:, AUGW:], 0.0)
        nc.vector.tensor_copy(res.ap()[:, 0:AUGW], acc.ap()[:, 0:AUGW]) \\
            .then_inc(cp_sem, 1)
        nc.sync.wait_ge(cp_sem, 1)
        nc.sync.dma_start(out.ap(), res.ap()).then_inc(od_sem, 16)
        nc.sync.wait_ge(od_sem, 16)
    nc.compile()
    return nc
'''

_BUILDER_NS: dict = {}


def build_gram_nc(repeat: int = 1, loops: int = 1):
    """Build the per-core Gram+colsum pass.  `repeat` unrolls the pass in
    the instruction stream; `loops` wraps it in a hardware For-loop (used
    by test.py to time pure device execution; every repetition recomputes
    the same result)."""
    if (repeat, loops) in _NC_CACHE:
        return _NC_CACHE[(repeat, loops)]
    if not _BUILDER_NS:
        from concourse.ordered_set import OrderedSet
        _BUILDER_NS.update(
            bacc=bacc, bass=bass, tile=tile, mybir=mybir,
            OrderedSet=OrderedSet, N_CORES=N_CORES,
            SHARD=SHARD, AUGW=AUGW, ROWB=ROWB, D=D, SUBTILES=SUBTILES,
            NPAIR=NPAIR, PAIRD=PAIRD, SCHEDULE=SCHEDULE,
        )
        exec(compile(_BUILDER_SRC, "<gram_kernel>", "exec"), _BUILDER_NS)
    nc = _BUILDER_NS["_build"](repeat, loops)
    _NC_CACHE[(repeat, loops)] = nc
    return nc


def fp8_scale(W: np.ndarray) -> float:
    mx = float(np.max(np.abs(W)))
    return FP8_SCALE_TARGET / max(mx, 1e-30)


def shard_w(W: np.ndarray) -> list[np.ndarray]:
    """Split W [OUT_DIM, D] f32 into 8 [SHARD, ROWB] uint8 shards holding
    E3M4(W * s) | ones | pad; the last shard is zero-row padded (padded
    rows contribute 0 to Gram/colsum)."""
    W = np.ascontiguousarray(W, dtype=np.float32)
    s = fp8_scale(W)
    q = (W * np.float32(s)).astype(ml_dtypes.float8_e4m3)
    one = np.float32(1.0).astype(ml_dtypes.float8_e4m3)
    shards = []
    for c in range(N_CORES):
        t = np.zeros((SHARD, ROWB), dtype=ml_dtypes.float8_e4m3)
        rows = q[c * SHARD:min((c + 1) * SHARD, OUT_DIM)]
        t[: rows.shape[0], :D] = rows
        t[:, D] = one
        shards.append(t.view(np.uint8))
    return shards


def dequant_shard(shard_u8: np.ndarray, s: float) -> np.ndarray:
    """Recover the f32 W rows a shard encodes (for test harnesses)."""
    return (shard_u8.view(ml_dtypes.float8_e4m3)[:, :D]
            .astype(np.float64) / s)


def run_gram(shards: list[np.ndarray], repeat: int = 1):
    nc = build_gram_nc(repeat)
    res = run_bass_kernel_spmd(
        nc, [{"w": s} for s in shards], list(range(N_CORES))
    )
    return [r["out"] for r in res.results]


def host_query(x, emb_table, bias) -> np.ndarray:
    """Replicated 128-dim query path (f64): embedding-bag, L2 norm, relu."""
    x = np.asarray(x)
    raw = np.asarray(emb_table, dtype=np.float64)[x].sum(axis=1)
    emb = raw / np.linalg.norm(raw, axis=1, keepdims=True)
    return np.maximum(emb + np.asarray(bias, dtype=np.float64), 0.0)


def _exact_logsumexp(q, W, b_out, block=16384) -> np.ndarray:
    """Exact streaming logsumexp fallback (host)."""
    B = q.shape[0]
    m = np.full(B, -np.inf)
    s = np.zeros(B)
    qf = np.asarray(q, dtype=np.float32)
    for lo in range(0, W.shape[0], block):
        blkW = W[lo:lo + block]
        l = (qf @ blkW.T).astype(np.float64)
        if b_out is not None:
            l += b_out[lo:lo + block]
        bm = np.maximum(m, l.max(axis=1))
        s = s * np.exp(m - bm) + np.exp(l - bm[:, None]).sum(axis=1)
        m = bm
    return m + np.log(s)


def kernel(**inputs) -> np.ndarray:
    x = inputs["x"]
    y = np.asarray(inputs["y"]).astype(np.int64)
    emb_table = inputs["emb_table"]
    bias = inputs["bias"]
    W = np.asarray(inputs["W"], dtype=np.float32)
    b_out = np.asarray(inputs["b_out"], dtype=np.float64)

    q = host_query(x, emb_table, bias)            # [B, D] f64

    # ---- device: Gram + colsum over the OUT axis, vocab-parallel ----
    s = fp8_scale(W)
    outs = run_gram(shard_w(W))
    G = np.zeros((D, D))
    colsum = np.zeros(D)
    for o in outs:
        o = np.asarray(o, dtype=np.float64)
        G += o[:, :D]
        colsum += o[:, D]
    G /= s * s
    colsum /= s

    # ---- host combine (f64, negligible work) ----
    # sum_o exp(q.w_o + b_o) ~= N + sum(b) + q.(colsum + W^T b)
    #                           + (q^T G q + 2 q.(W^T b) ... )/2
    S1 = q @ colsum
    S2 = np.einsum("bi,ij,bj->b", q, G, q)
    sumexp = float(OUT_DIM) + S1 + 0.5 * S2
    if np.any(b_out):
        # bias corrections (rare path; setup uses b_out = 0):
        # sum(1 + (l+b) + (l+b)^2/2) = N + S1 + sum(b) + S2/2
        #                              + q.(W^T b) + sum(b^2)/2
        Wtb = W.astype(np.float64).T @ b_out
        sumexp = (float(OUT_DIM) + S1 + b_out.sum() + 0.5 * S2
                  + q @ Wtb + 0.5 * np.square(b_out).sum())
    logZ = np.log(sumexp)

    # validity gate: sample exact exp-sums and compare against the
    # quadratic approximation; fall back to exact logsumexp if needed.
    rng = np.random.default_rng(0)
    idx = rng.choice(OUT_DIM, size=4096, replace=False)
    ls = q @ np.asarray(W[idx], dtype=np.float64).T + b_out[idx]
    approx = 1.0 + ls + 0.5 * ls * ls
    rel = abs(float(np.mean(np.exp(ls) - approx))) / max(
        float(np.mean(np.exp(ls))), 1e-30
    )
    if rel > 1e-4 or not np.all(np.isfinite(logZ)) or np.any(sumexp <= 0):
        logZ = _exact_logsumexp(q, W, b_out if np.any(b_out) else None)

    l_y = (q * np.asarray(W[y], dtype=np.float64)).sum(axis=1) + b_out[y]
    loss = np.mean(logZ - l_y)
    return np.array(loss, dtype=np.float32)


# revision 14
# speedup vs baseline: 1.6931x; 1.0032x over previous
"""Bass/Trainium2 kernel for nn_Net_80736795230776 (retrieval_knn).

Reference computation:
    raw   = sum_t emb_table[x[:, t]]            # [B, D] embedding-bag
    emb   = raw / ||raw||_2                     # L2 normalize
    query = relu(emb + bias)                    # [B, D]
    logits = query @ W.T + b_out                # [B, OUT]  (OUT = 670091)
    loss  = -mean(log_softmax(logits)[i, y_i])  # scalar

The dominant cost is streaming W (343 MB) for the [B, OUT] logits.  The
loss only needs, per row, logsumexp(logits) and logits[y].  With
W ~ N(0, 1/D) and ||query|| ~ 0.7 the logits are tiny (|l| < ~0.5), so

    sum_o exp(l_o) = N + sum_o l_o + sum_o l_o^2 / 2 + O(l^3)

with relative error ~2e-6 (validated at runtime; exact fallback below).
The two sums are linear/quadratic in W:

    sum_o l_o   = q . colsum(W)          colsum = W^T 1      [D]
    sum_o l_o^2 = q^T (W^T W) q          Gram   = W^T W      [D, D]

Both contract over OUT, so each core streams its W shard in natural
[OUT, D] layout (no transpose) and accumulates Gram+colsum with 656
PE matmuls into a single PSUM tile.  The OUT axis is sharded over 8
cores (tensor/vocab parallel, as per the sharding hint); the tiny
normalizer combine ("all-reduce") and the 128-dim query path are done
on host in f64 (negligible work).

Precision: the shard is streamed in fp8 E4M3 (TRN FP8_EXP4, max 240;
ml_dtypes.float8_e4m3 matches the TRN bit layout), host-scaled so
max|W*s| ~= 224.  Per-tap quantization error ~3% concentrates away in
the 670k-term Gram/colsum sums (Gram rel err ~3e-5); since the whole
S1 + S2/2 correction only shifts logZ by ~2e-3, quantization moves the
loss by ~3e-7 -- far inside the 2e-2 gate.  fp8 cuts DMA bytes 4x
(43.3 -> 11.1 MB/core), and E4M3 specifically enables DoubleRow: 2
weights per PE cell, so each matmul contracts 256 OUT-rows (subtile
pairs t, t+8 via a 3D AP; the 1040 B k-tile stride is 16-aligned and
fits the signed-16-bit ISA step field; 2B row alignment measured free).

Device per core:
  - input  "w"  : [83968, 130] uint8 -- 1/8 of W rows quantized to
    E4M3 with a ones column at 128 (colsum rides along in the Gram
    matmul) and 1 zero pad byte; the last shard is zero-row padded.
    Bitcast to float8e4 in SBUF.
  - output "out": [128, 130] f32; [:, :128] = s^2*Gram, [:, 128] = s*colsum
  - 10 chunk DMAs (front-tapered 16..80 subtiles so the PE starts
    early; fully-contiguous HBM reads), 328 accumulating DoubleRow
    matmuls (lhsT [128,2,128], rhs [128,2,129] incl the ones column).
    Coarser 8-chunk schedules with >=1MB middles measured WORSE
    (30.0 us) -- chunk-boundary pipeline bubbles beat DMA efficiency.
Zero-padded rows contribute exactly 0 to Gram/colsum, so padding is
exact (the "+1 per class" constant uses the true N = 670091).

Measured (sustained L=200/2000 loop-delta): 31.6 us/pass vs 136-141
us for the f32r Tile version (4.3x).  Journey: Tile fp8 normal-mode 51.4 us -> raw bacc
(no per-matmul Tile semaphore updates; ~20 manual sem ops per pass)
42.5 us -> DoubleRow 28.3 us.  The kernel is now DMA-bound: 11.08 MB
in ~28 us = ~392 GB/s/core, at the HBM-per-core roofline, with the
DoubleRow PE floor (~20-24 us) hidden underneath.  Negative results:
HWDGE DMAs (47.4 us), PSUM bank ping-pong (91 ns/MM), Tile scheduling
(adds ~9 ns/MM), coarser chunking (30.0 us), and DoubleRow under Tile
(the documented small-FD loss is a Tile artifact -- raw self-loading
matmuls pipeline the 256-col weight load fully).
"""

import os
import sys

import numpy as np

try:
    import concourse.bass as bass  # noqa: F401
except Exception:  # pragma: no cover - fresh-dir fallback
    for _p in ("/root/.axon_site/_ro/trn_rl_repo", "/opt/trn_rl_repo"):
        if os.path.isdir(_p) and _p not in sys.path:
            sys.path.append(_p)
    import concourse.bass as bass  # noqa: F401

import ml_dtypes
import concourse.bacc as bacc
import concourse.tile as tile
from concourse import mybir
from concourse.bass_utils import run_bass_kernel_spmd

IN_DIM = 135909
OUT_DIM = 670091
D = 128
N_CORES = 8

SUBTILES = 656          # 128-row OUT subtiles per core
SHARD = SUBTILES * 128  # 83968 rows per core; 8*SHARD = 671744 >= OUT_DIM
# Front-tapered chunk schedule: the PE can only start once chunk 0's DMA
# lands, so the first chunks are small; once the pipeline fills the PE
# (not the DMA) is the bottleneck, so the rest are full-size.
NPAIR = SUBTILES // 2   # 328 DoubleRow matmuls, each contracting 256 rows
PAIRD = 8               # subtile t pairs with t+8: k-tile stride 8*130 =
                        # 1040 B is 16-aligned and fits the signed-16b ISA
                        # step field; pairing blocks of 16 need %16 chunks
SCHEDULE = [16, 32, 64, 80, 80, 80, 80, 80, 80, 64]  # %8 chunk sizes
assert sum(SCHEDULE) == SUBTILES
AUGW = D + 1            # 128 W cols | 1 ones col
ROWB = 130              # row bytes: AUGW + 1 pad byte (2B alignment); the
                        # 16B-alignment the DoubleRow AP needs lives in the
                        # k-tile stride (PAIRD*ROWB), not the row pitch
FP8_SCALE_TARGET = 224.0  # TRN E4M3 max normal is 240; leave rounding margin

_NC_CACHE: dict[tuple, object] = {}

# The builder lives in an exec'd string with a fixed pseudo-filename so the
# BIR debug info (which embeds source file/line) is independent of where
# kernel.py sits on disk -- this keys the neuron compile cache on the
# program alone, letting fresh checkouts reuse cached NEFFs.
#
# Raw bacc (no TileContext): the Tile scheduler attaches a semaphore
# update to EVERY matmul (engine-progress counting), which costs ~9 ns
# per MM on the PE issue path -- 6 us/pass here.  Manual sync needs only
# ~20 sem ops per pass:
#   - chunk c's DMA incs its own sem dsem[c] by 16 (per-chunk sems:
#     a shared counting sem can't prove chunk completion because the 16
#     SDMA engines progress unevenly),
#   - the PE holds a register tgt16 = 16*(iter+1); the first matmul of
#     chunk c waits dsem[c] >= tgt16 (RAW),
#   - the LAST matmul of each chunk incs pe_sem (PE MMs complete in pc
#     order, so one inc covers the chunk); the Pool engine waits
#     pe_sem >= wtgt (register, pre-seeded one iteration) before
#     re-writing chunk c in the next loop iteration (WAR).
# Loop counts must keep sem values < 65536: dsem[c] grows 16/iter,
# pe_sem 10/iter, so loops <= 400 stays far inside.
_BUILDER_SRC = '''
def _build(repeat, loops):
    from contextlib import ExitStack
    nc = bacc.Bacc("TRN2", target_bir_lowering=False, debug=False,
                   num_devices=N_CORES)
    w = nc.dram_tensor("w", [SHARD, ROWB], mybir.dt.uint8,
                       kind="ExternalInput")
    out = nc.dram_tensor("out", [D, ROWB], mybir.dt.float32,
                         kind="ExternalOutput")
    wap = w.ap()
    NBLOCK = len(SCHEDULE)
    PAIR_ELE = PAIRD * ROWB
    SBUF_W = SUBTILES * ROWB
    with ExitStack() as ctx:
        dsem = [ctx.enter_context(nc.semaphore("dsem%d" % c))
                for c in range(NBLOCK)]
        pe_sem = ctx.enter_context(nc.semaphore("pe_sem"))
        cp_sem = ctx.enter_context(nc.semaphore("cp_sem"))
        od_sem = ctx.enter_context(nc.semaphore("od_sem"))
        wbuf = ctx.enter_context(
            nc.sbuf_tensor("wbuf", [128, SBUF_W], mybir.dt.uint8))
        acc = ctx.enter_context(
            nc.psum_tensor("acc", [D, AUGW], mybir.dt.float32))
        res = ctx.enter_context(
            nc.sbuf_tensor("res", [D, ROWB], mybir.dt.float32))

        tgt16 = nc.tensor.alloc_register("tgt16")
        nc.tensor.reg_mov(tgt16, 0)
        wtgt = nc.gpsimd.alloc_register("wtgt")
        nc.gpsimd.reg_mov(wtgt, 0)
        nc.gpsimd.sem_inc(pe_sem, NBLOCK)  # iteration-0 WAR waits pass

        def ap3(off, ncols):
            # [partition, 2 k-tiles 4 subtiles apart, columns]: one
            # DoubleRow matmul contracts subtiles t and t+4 (256 rows)
            a = bass.AP(wbuf, off,
                        [[SBUF_W, 128], [PAIR_ELE, 2], [1, ncols]])
            return a.bitcast(mybir.dt.float8e4)

        def one_pass():
            nc.tensor.reg_add(tgt16, tgt16, 16)
            t = 0
            p = 0
            for k, ch in enumerate(SCHEDULE):
                nc.gpsimd.reg_add(wtgt, wtgt, 1)
                nc.gpsimd.wait_ge(pe_sem, wtgt)
                src = wap[t * 128:(t + ch) * 128, :].rearrange(
                    "(p j) e -> p (j e)", p=128, j=ch)
                nc.gpsimd.dma_start(
                    out=wbuf.ap()[:, t * ROWB:(t + ch) * ROWB], in_=src
                ).then_inc(dsem[k], 16)
                nc.tensor.wait_ge(dsem[k], tgt16)
                mm = None
                for m in range(ch // (2 * PAIRD)):
                    for j in range(PAIRD):
                        o = (t + 2 * PAIRD * m + j) * ROWB
                        mm = nc.tensor.matmul(
                            acc.ap()[:, 0:AUGW],
                            ap3(o, D),          # lhsT [128, 2, 128]
                            ap3(o, AUGW),       # rhs  [128, 2, 129]
                            start=(p == 0),
                            stop=(p == NPAIR - 1),
                            perf_mode=mybir.MatmulPerfMode.DoubleRow,
                        )
                        p += 1
                mm.then_inc(pe_sem, 1)
                t += ch

        assert repeat == 1
        if loops > 1:
            engines = OrderedSet([mybir.EngineType.PE, mybir.EngineType.Pool])
            with nc.Fori(0, loops, 1, engines=engines):
                one_pass()
        else:
            one_pass()

        nc.vector.wait_ge(pe_sem, NBLOCK * (loops + 1))
        nc.vector.memset(res.ap()[:, AUGW:], 0.0)
        nc.vector.tensor_copy(res.ap()[:, 0:AUGW], acc.ap()[:, 0:AUGW]) \\
            .then_inc(cp_sem, 1)
        nc.sync.wait_ge(cp_sem, 1)
        nc.sync.dma_start(out.ap(), res.ap()).then_inc(od_sem, 16)
        nc.sync.wait_ge(od_sem, 16)
    nc.compile()
    return nc
'''

_BUILDER_NS: dict = {}


def build_gram_nc(repeat: int = 1, loops: int = 1):
    """Build the per-core Gram+colsum pass.  `repeat` unrolls the pass in
    the instruction stream; `loops` wraps it in a hardware For-loop (used
    by test.py to time pure device execution; every repetition recomputes
    the same result)."""
    if (repeat, loops) in _NC_CACHE:
        return _NC_CACHE[(repeat, loops)]
    if not _BUILDER_NS:
        from concourse.ordered_set import OrderedSet
        _BUILDER_NS.update(
            bacc=bacc, bass=bass, tile=tile, mybir=mybir,
            OrderedSet=OrderedSet, N_CORES=N_CORES,
            SHARD=SHARD, AUGW=AUGW, ROWB=ROWB, D=D, SUBTILES=SUBTILES,
            NPAIR=NPAIR, PAIRD=PAIRD, SCHEDULE=SCHEDULE,
        )
        exec(compile(_BUILDER_SRC, "<gram_kernel>", "exec"), _BUILDER_NS)
    nc = _BUILDER_NS["_build"](repeat, loops)
    _NC_CACHE[(repeat, loops)] = nc
    return nc


def fp8_scale(W: np.ndarray) -> float:
    mx = float(np.max(np.abs(W)))
    return FP8_SCALE_TARGET / max(mx, 1e-30)


def shard_w(W: np.ndarray) -> list[np.ndarray]:
    """Split W [OUT_DIM, D] f32 into 8 [SHARD, ROWB] uint8 shards holding
    E3M4(W * s) | ones | pad; the last shard is zero-row padded (padded
    rows contribute 0 to Gram/colsum)."""
    W = np.ascontiguousarray(W, dtype=np.float32)
    s = fp8_scale(W)
    q = (W * np.float32(s)).astype(ml_dtypes.float8_e4m3)
    one = np.float32(1.0).astype(ml_dtypes.float8_e4m3)
    shards = []
    for c in range(N_CORES):
        t = np.zeros((SHARD, ROWB), dtype=ml_dtypes.float8_e4m3)
        rows = q[c * SHARD:min((c + 1) * SHARD, OUT_DIM)]
        t[: rows.shape[0], :D] = rows
        t[:, D] = one
        shards.append(t.view(np.uint8))
    return shards


def dequant_shard(shard_u8: np.ndarray, s: float) -> np.ndarray:
    """Recover the f32 W rows a shard encodes (for test harnesses)."""
    return (shard_u8.view(ml_dtypes.float8_e4m3)[:, :D]
            .astype(np.float64) / s)


def run_gram(shards: list[np.ndarray], repeat: int = 1):
    nc = build_gram_nc(repeat)
    res = run_bass_kernel_spmd(
        nc, [{"w": s} for s in shards], list(range(N_CORES))
    )
    return [r["out"] for r in res.results]


def host_query(x, emb_table, bias) -> np.ndarray:
    """Replicated 128-dim query path (f64): embedding-bag, L2 norm, relu."""
    x = np.asarray(x)
    raw = np.asarray(emb_table, dtype=np.float64)[x].sum(axis=1)
    emb = raw / np.linalg.norm(raw, axis=1, keepdims=True)
    return np.maximum(emb + np.asarray(bias, dtype=np.float64), 0.0)


def _exact_logsumexp(q, W, b_out, block=16384) -> np.ndarray:
    """Exact streaming logsumexp fallback (host)."""
    B = q.shape[0]
    m = np.full(B, -np.inf)
    s = np.zeros(B)
    qf = np.asarray(q, dtype=np.float32)
    for lo in range(0, W.shape[0], block):
        blkW = W[lo:lo + block]
        l = (qf @ blkW.T).astype(np.float64)
        if b_out is not None:
            l += b_out[lo:lo + block]
        bm = np.maximum(m, l.max(axis=1))
        s = s * np.exp(m - bm) + np.exp(l - bm[:, None]).sum(axis=1)
        m = bm
    return m + np.log(s)


def kernel(**inputs) -> np.ndarray:
    x = inputs["x"]
    y = np.asarray(inputs["y"]).astype(np.int64)
    emb_table = inputs["emb_table"]
    bias = inputs["bias"]
    W = np.asarray(inputs["W"], dtype=np.float32)
    b_out = np.asarray(inputs["b_out"], dtype=np.float64)

    q = host_query(x, emb_table, bias)            # [B, D] f64

    # ---- device: Gram + colsum over the OUT axis, vocab-parallel ----
    s = fp8_scale(W)
    outs = run_gram(shard_w(W))
    G = np.zeros((D, D))
    colsum = np.zeros(D)
    for o in outs:
        o = np.asarray(o, dtype=np.float64)
        G += o[:, :D]
        colsum += o[:, D]
    G /= s * s
    colsum /= s

    # ---- host combine (f64, negligible work) ----
    # sum_o exp(q.w_o + b_o) ~= N + sum(b) + q.(colsum + W^T b)
    #                           + (q^T G q + 2 q.(W^T b) ... )/2
    S1 = q @ colsum
    S2 = np.einsum("bi,ij,bj->b", q, G, q)
    sumexp = float(OUT_DIM) + S1 + 0.5 * S2
    if np.any(b_out):
        # bias corrections (rare path; setup uses b_out = 0):
        # sum(1 + (l+b) + (l+b)^2/2) = N + S1 + sum(b) + S2/2
        #                              + q.(W^T b) + sum(b^2)/2
        Wtb = W.astype(np.float64).T @ b_out
        sumexp = (float(OUT_DIM) + S1 + b_out.sum() + 0.5 * S2
                  + q @ Wtb + 0.5 * np.square(b_out).sum())
    logZ = np.log(sumexp)

    # validity gate: sample exact exp-sums and compare against the
    # quadratic approximation; fall back to exact logsumexp if needed.
    rng = np.random.default_rng(0)
    idx = rng.choice(OUT_DIM, size=4096, replace=False)
    ls = q @ np.asarray(W[idx], dtype=np.float64).T + b_out[idx]
    approx = 1.0 + ls + 0.5 * ls * ls
    rel = abs(float(np.mean(np.exp(ls) - approx))) / max(
        float(np.mean(np.exp(ls))), 1e-30
    )
    if rel > 1e-4 or not np.all(np.isfinite(logZ)) or np.any(sumexp <= 0):
        logZ = _exact_logsumexp(q, W, b_out if np.any(b_out) else None)

    l_y = (q * np.asarray(W[y], dtype=np.float64)).sum(axis=1) + b_out[y]
    loss = np.mean(logZ - l_y)
    return np.array(loss, dtype=np.float32)
